# revision 1
# baseline (speedup 1.0000x reference)
"""HaarMSELoss kernel for Trainium2 (8 NeuronCores, data-parallel).

Math: the 2x2 Haar transform used by the reference is (up to the 0.5
scaling) an orthogonal Hadamard transform, so for each 2x2 block
LL^2+LH^2+HL^2+HH^2 == a^2+b^2+c^2+d^2 of the block entries of
(input - target).  Hence

  loss = sum_bands mean((haar(x)-haar(y))^2)
       = sum((x-y)^2) / (B*C*(H/2)*(W/2))

i.e. a pure squared-difference reduction.  Each core reduces 1/8 of the
elements; the host sums the 8x128 per-partition partials (f64) and
divides.

Layout: per core the two chunks are interleaved host-side into one
[128, 2, FREE] array (row p = x-row p, y-row p) so each SBUF tile of
both operands arrives with a single dma_start.

Raw bass pipeline (Tile's auto-sems exceed this walrus build's 3-waits-
per-instruction ISA limit, so sems are explicit; every instruction
waits on at most one semaphore):
  SP  : dma loads (slot-recycled against ACT), final stats store
  DVE : d = x - y in place
  ACT : stats[:,t] = sum(d^2) via activation(Square, accum_out)
"""

import numpy as np

_B, _C, _H, _W = 4, 32, 512, 512
_TOTAL = _B * _C * _H * _W          # 33_554_432
_NCORES = 8
_PER_CORE = _TOTAL // _NCORES       # 4_194_304
_P = 128
_FREE = _PER_CORE // _P             # 32_768 f32 per partition per tensor
_F = 4096                           # tile free dim per operand (4 MiB / DMA)
_T = _FREE // _F                    # 8 tiles
_NBUF = 3
_DIVISOR = float(_TOTAL // 4)       # 8_388_608  (elements per subband)

_CACHE = {}


def _build_nc():
    from contextlib import ExitStack
    import concourse.bass as bass
    import concourse.mybir as mybir

    f32 = mybir.dt.float32
    nc = bass.Bass("TRN2", target_bir_lowering=False)
    xy = nc.dram_tensor("xy", [_P, 2, _FREE], f32, kind="ExternalInput")
    out = nc.dram_tensor("out", [_P, _T], f32, kind="ExternalOutput")

    ctx = ExitStack()
    nc._ctx = ctx  # keep SBUF/semaphore handles alive for compile
    slots = [ctx.enter_context(nc.sbuf_tensor(f"slot{i}", [_P, 2, _F], f32))
             for i in range(_NBUF)]
    stats = ctx.enter_context(nc.sbuf_tensor([_P, _T], f32))
    zbias = ctx.enter_context(nc.sbuf_tensor([_P, 1], f32))
    dma_sem = ctx.enter_context(nc.semaphore())
    dve_sem = ctx.enter_context(nc.semaphore())
    act_sem = ctx.enter_context(nc.semaphore())
    block = ctx.enter_context(nc.Block())

    @block.sync
    def _(sync):
        for t in range(_T):
            if t >= _NBUF:
                # slot free once ACT (last reader) finished tile t-NBUF
                sync.wait_ge(act_sem, t - _NBUF + 1)
            sync.dma_start(
                out=slots[t % _NBUF][:], in_=xy[:, :, t * _F:(t + 1) * _F]
            ).then_inc(dma_sem, 16)
        sync.wait_ge(act_sem, _T)
        sync.dma_start(out=out[:], in_=stats[:]).then_inc(dma_sem, 16)
        sync.wait_ge(dma_sem, 16 * (_T + 1))  # store landed

    @block.vector
    def _(vector):
        vector.memset(zbias[:], 0.0).then_inc(dve_sem, 1)
        for t in range(_T):
            vector.wait_ge(dma_sem, 16 * (t + 1))
            st = slots[t % _NBUF]
            vector.tensor_sub(st[:, 0, :], st[:, 0, :], st[:, 1, :]) \
                  .then_inc(dve_sem, 1)

    @block.scalar
    def _(scalar):
        for t in range(_T):
            scalar.wait_ge(dve_sem, t + 2)
            st = slots[t % _NBUF]
            scalar.activation(
                st[:, 0, :], st[:, 0, :], mybir.ActivationFunctionType.Square,
                bias=zbias[:, 0:1], accum_out=stats[:, t:t + 1],
            ).then_inc(act_sem, 1)

    ctx.close()
    return nc


def _run(in_maps, trace=False):
    from concourse.bass_utils import run_bass_kernel_spmd

    if "nc" not in _CACHE:
        _CACHE["nc"] = _build_nc()
    return run_bass_kernel_spmd(
        _CACHE["nc"], in_maps, list(range(_NCORES)), trace=trace
    )


def _make_in_maps(input, target):
    xs = np.asarray(input, dtype=np.float32).reshape(_NCORES, _P, _FREE)
    ys = np.asarray(target, dtype=np.float32).reshape(_NCORES, _P, _FREE)
    maps = []
    for c in range(_NCORES):
        xy = np.empty((_P, 2, _FREE), dtype=np.float32)
        xy[:, 0, :] = xs[c]
        xy[:, 1, :] = ys[c]
        maps.append({"xy": xy})
    return maps


def _finish(results):
    total = 0.0
    for r in results:
        total += r["out"].astype(np.float64).sum()
    return np.array(total / _DIVISOR, dtype=np.float32)


def kernel(input, target):
    res = _run(_make_in_maps(input, target), trace=False)
    return _finish(res.results)



# revision 5
# speedup vs baseline: 1.1902x; 1.1902x over previous
"""HaarMSELoss kernel for Trainium2 (8 NeuronCores, data-parallel).

Math: the 2x2 Haar transform used by the reference is (up to the 0.5
scaling) an orthogonal Hadamard transform, so for each 2x2 block
LL^2+LH^2+HL^2+HH^2 == a^2+b^2+c^2+d^2 of the block entries of
(input - target).  Hence

  loss = sum_bands mean((haar(x)-haar(y))^2)
       = sum((x-y)^2) / (B*C*(H/2)*(W/2))

i.e. a pure squared-difference reduction.  Each core reduces 1/8 of the
elements; the host sums the 8x128 per-partition partials (f64) and
divides.

Layout: per core the two chunks are interleaved host-side into one
[128, 2, FREE] array (row p = x-row p, y-row p) so each SBUF tile of
both operands arrives with a single dma_start.

Pipeline (raw bass; every instruction waits on at most one semaphore,
the walrus 3-waits-per-instruction ISA limit):
  SP  : dma loads, final stats store
  DVE : d = x - y in place
  ACT : stats[:,t] = sum(d^2) via activation(Square, accum_out)

Tiling: seven 4 MiB transfers for bandwidth, then a tapered tail
(2048/1024/512/256/256 f32 per partition) so the post-last-byte serial
chain (DVE sub -> ACT square -> store) runs on 256 elems instead of
4096.  HWDGE transfers on one queue complete OUT OF ORDER when sizes
differ, so a single shared dma semaphore miscounts: each big slot has
its own semaphore (slot reuse is ACT-gated, so within a slot the
count is race-free), and each taper tile lands in fresh SBUF with its
own semaphore waited to exactly 16.
"""

import numpy as np

_B, _C, _H, _W = 4, 32, 512, 512
_TOTAL = _B * _C * _H * _W          # 33_554_432
_NCORES = 8
_PER_CORE = _TOTAL // _NCORES       # 4_194_304
_P = 128
_FREE = _PER_CORE // _P             # 32_768 f32 per partition per tensor
_F = 4096                           # big tile free dim (4 MiB / DMA)
_NBIG = 4                           # big slot count (recycled)
_SIZES = [4096] * 7 + [2048, 1024, 512, 256, 256]
assert sum(_SIZES) == _FREE
_T = len(_SIZES)                    # 12 tiles
_OFFS = [sum(_SIZES[:i]) for i in range(_T)]
_NTAPER = 4                         # tiles 8..11 -> dedicated taper slot
_TSIZES = _SIZES[8:]                # [1024, 512, 256, 256]
_TOFF = [sum(_TSIZES[:i]) for i in range(_NTAPER)]
_TLEN = sum(_TSIZES)                # 2048
_DIVISOR = float(_TOTAL // 4)       # 8_388_608  (elements per subband)

_CACHE = {}


def _build_nc():
    from contextlib import ExitStack
    import concourse.bass as bass
    import concourse.mybir as mybir

    f32 = mybir.dt.float32
    nc = bass.Bass("TRN2", target_bir_lowering=False)
    xy = nc.dram_tensor("xy", [_P, 2, _FREE], f32, kind="ExternalInput")
    out = nc.dram_tensor("out", [_P, _T], f32, kind="ExternalOutput")

    ctx = ExitStack()
    nc._ctx = ctx  # keep SBUF/semaphore handles alive for compile
    big = [ctx.enter_context(nc.sbuf_tensor(f"big{i}", [_P, 2, _F], f32))
           for i in range(_NBIG)]
    taper = ctx.enter_context(nc.sbuf_tensor("taper", [_P, 2, _TLEN], f32))
    stats = ctx.enter_context(nc.sbuf_tensor([_P, _T], f32))
    zbias = ctx.enter_context(nc.sbuf_tensor([_P, 1], f32))
    slot_sem = [ctx.enter_context(nc.semaphore(f"slot_sem{i}"))
                for i in range(_NBIG)]
    tile_sem = [ctx.enter_context(nc.semaphore(f"tile_sem{i}"))
                for i in range(_NTAPER)]
    dve_sem = ctx.enter_context(nc.semaphore("dve_sem"))
    act_sem = ctx.enter_context(nc.semaphore("act_sem"))
    store_sem = ctx.enter_context(nc.semaphore("store_sem"))
    block = ctx.enter_context(nc.Block())

    def tile_ap(t):
        # (x-row AP, y-row AP) for tile t
        f = _SIZES[t]
        if t < 8:
            st = big[t % _NBIG]
            return st[:, 0, :f], st[:, 1, :f]
        o = _TOFF[t - 8]
        return taper[:, 0, o:o + f], taper[:, 1, o:o + f]

    def land_wait(eng, t):
        # wait until tile t's DMA fully landed
        if t < 8:
            eng.wait_ge(slot_sem[t % _NBIG], 16 * (t // _NBIG + 1))
        else:
            eng.wait_ge(tile_sem[t - 8], 16)

    @block.sync
    def _(sync):
        for t in range(_T):
            if 8 > t >= _NBIG:
                # slot free once ACT (last reader) finished tile t-NBIG
                sync.wait_ge(act_sem, t - _NBIG + 1)
            f, off = _SIZES[t], _OFFS[t]
            if t < 8:
                dst = big[t % _NBIG][:, :, :f]
                sem = slot_sem[t % _NBIG]
            else:
                o = _TOFF[t - 8]
                dst = taper[:, :, o:o + f]
                sem = tile_sem[t - 8]
            sync.dma_start(out=dst, in_=xy[:, :, off:off + f]).then_inc(sem, 16)
        sync.wait_ge(act_sem, _T)
        sync.dma_start(out=out[:], in_=stats[:]).then_inc(store_sem, 16)
        sync.wait_ge(store_sem, 16)  # store landed

    @block.vector
    def _(vector):
        vector.memset(zbias[:], 0.0).then_inc(dve_sem, 1)
        for t in range(_T):
            land_wait(vector, t)
            a, b = tile_ap(t)
            vector.tensor_sub(a, a, b).then_inc(dve_sem, 1)

    @block.scalar
    def _(scalar):
        for t in range(_T):
            scalar.wait_ge(dve_sem, t + 2)
            a, _ = tile_ap(t)
            scalar.activation(
                a, a, mybir.ActivationFunctionType.Square,
                bias=zbias[:, 0:1], accum_out=stats[:, t:t + 1],
            ).then_inc(act_sem, 1)

    ctx.close()
    return nc


def _run(in_maps, trace=False):
    from concourse.bass_utils import run_bass_kernel_spmd

    if "nc" not in _CACHE:
        _CACHE["nc"] = _build_nc()
    return run_bass_kernel_spmd(
        _CACHE["nc"], in_maps, list(range(_NCORES)), trace=trace
    )


def _make_in_maps(input, target):
    xs = np.asarray(input, dtype=np.float32).reshape(_NCORES, _P, _FREE)
    ys = np.asarray(target, dtype=np.float32).reshape(_NCORES, _P, _FREE)
    maps = []
    for c in range(_NCORES):
        xy = np.empty((_P, 2, _FREE), dtype=np.float32)
        xy[:, 0, :] = xs[c]
        xy[:, 1, :] = ys[c]
        maps.append({"xy": xy})
    return maps


def _finish(results):
    total = 0.0
    for r in results:
        total += r["out"].astype(np.float64).sum()
    return np.array(total / _DIVISOR, dtype=np.float32)


def kernel(input, target):
    res = _run(_make_in_maps(input, target), trace=False)
    return _finish(res.results)


# revision 6
# speedup vs baseline: 1.2151x; 1.0209x over previous
"""HaarMSELoss kernel for Trainium2 (8 NeuronCores, data-parallel).

Math: the 2x2 Haar transform used by the reference is (up to the 0.5
scaling) an orthogonal Hadamard transform, so for each 2x2 block
LL^2+LH^2+HL^2+HH^2 == a^2+b^2+c^2+d^2 of the block entries of
(input - target).  Hence

  loss = sum_bands mean((haar(x)-haar(y))^2)
       = sum((x-y)^2) / (B*C*(H/2)*(W/2))

i.e. a pure squared-difference reduction.  Each core reduces 1/8 of the
elements; the host sums the 8x128 per-partition partials (f64) and
divides.

Layout: per core the two chunks are interleaved host-side into one
[128, 2, FREE] array (row p = x-row p, y-row p) so each SBUF tile of
both operands arrives with a single dma_start.

Pipeline (raw bass; every instruction waits on at most one semaphore,
the walrus 3-waits-per-instruction ISA limit):
  SP  : dma loads, final stats store
  DVE : d = x - y in place; fused d*d sum for two tail tiles
  ACT : stats[:,t] = sum(d^2) via activation(Square, accum_out)

Tiling: seven 4 MiB transfers for bandwidth, then a tapered tail
(3x1024 / 512 / 256 / 256 f32 per partition) so the post-last-byte
serial chain runs on small tiles.  The tail squares alternate between
ACT (activation Square) and DVE (scalar_tensor_tensor d*d with
accum_out) so the two engines drain the tail in parallel.

HWDGE transfers on one queue complete OUT OF ORDER when sizes differ,
so a single shared dma semaphore miscounts: each big slot has its own
semaphore (slot reuse is ACT-gated, so within a slot the count is
race-free), and each taper tile lands in fresh SBUF with its own
semaphore waited to exactly 16.
"""

import numpy as np

_B, _C, _H, _W = 4, 32, 512, 512
_TOTAL = _B * _C * _H * _W          # 33_554_432
_NCORES = 8
_PER_CORE = _TOTAL // _NCORES       # 4_194_304
_P = 128
_FREE = _PER_CORE // _P             # 32_768 f32 per partition per tensor
_F = 4096                           # big tile free dim (4 MiB / DMA)
_NBIG = 4                           # big slot count (recycled)
_NBIGT = 7                          # tiles 0..6 are big
_SIZES = [4096] * _NBIGT + [1024, 1024, 1024, 512, 256, 256]
assert sum(_SIZES) == _FREE
_T = len(_SIZES)                    # 13 tiles
_OFFS = [sum(_SIZES[:i]) for i in range(_T)]
_TSIZES = _SIZES[_NBIGT:]           # taper tile sizes
_NTAPER = len(_TSIZES)              # 6
_TOFF = [sum(_TSIZES[:i]) for i in range(_NTAPER)]
_TLEN = sum(_TSIZES)                # 4096
_DVE_SQ = (10, 12)                  # tail tiles squared on DVE, not ACT
_DIVISOR = float(_TOTAL // 4)       # 8_388_608  (elements per subband)

_CACHE = {}


def _build_nc():
    from contextlib import ExitStack
    import concourse.bass as bass
    import concourse.mybir as mybir

    f32 = mybir.dt.float32
    nc = bass.Bass("TRN2", target_bir_lowering=False)
    xy = nc.dram_tensor("xy", [_P, 2, _FREE], f32, kind="ExternalInput")
    out = nc.dram_tensor("out", [_P, _T], f32, kind="ExternalOutput")

    ctx = ExitStack()
    nc._ctx = ctx  # keep SBUF/semaphore handles alive for compile
    big = [ctx.enter_context(nc.sbuf_tensor(f"big{i}", [_P, 2, _F], f32))
           for i in range(_NBIG)]
    taper = ctx.enter_context(nc.sbuf_tensor("taper", [_P, 2, _TLEN], f32))
    stats = ctx.enter_context(nc.sbuf_tensor([_P, _T], f32))
    zbias = ctx.enter_context(nc.sbuf_tensor([_P, 1], f32))
    slot_sem = [ctx.enter_context(nc.semaphore(f"slot_sem{i}"))
                for i in range(_NBIG)]
    tile_sem = [ctx.enter_context(nc.semaphore(f"tile_sem{i}"))
                for i in range(_NTAPER)]
    dve_sem = ctx.enter_context(nc.semaphore("dve_sem"))
    act_sem = ctx.enter_context(nc.semaphore("act_sem"))
    store_sem = ctx.enter_context(nc.semaphore("store_sem"))
    block = ctx.enter_context(nc.Block())

    def tile_ap(t):
        # (x-row AP, y-row AP) for tile t
        f = _SIZES[t]
        if t < _NBIGT:
            st = big[t % _NBIG]
            return st[:, 0, :f], st[:, 1, :f]
        o = _TOFF[t - _NBIGT]
        return taper[:, 0, o:o + f], taper[:, 1, o:o + f]

    def land_wait(eng, t):
        # wait until tile t's DMA fully landed
        if t < _NBIGT:
            eng.wait_ge(slot_sem[t % _NBIG], 16 * (t // _NBIG + 1))
        else:
            eng.wait_ge(tile_sem[t - _NBIGT], 16)

    @block.sync
    def _(sync):
        for t in range(_T):
            if _NBIGT > t >= _NBIG:
                # slot free once ACT (last reader) finished tile t-NBIG
                sync.wait_ge(act_sem, t - _NBIG + 1)
            f, off = _SIZES[t], _OFFS[t]
            if t < _NBIGT:
                dst = big[t % _NBIG][:, :, :f]
                sem = slot_sem[t % _NBIG]
            else:
                o = _TOFF[t - _NBIGT]
                dst = taper[:, :, o:o + f]
                sem = tile_sem[t - _NBIGT]
            sync.dma_start(out=dst, in_=xy[:, :, off:off + f]).then_inc(sem, 16)
        sync.wait_ge(act_sem, _T)
        sync.dma_start(out=out[:], in_=stats[:]).then_inc(store_sem, 16)
        sync.wait_ge(store_sem, 16)  # store landed

    @block.vector
    def _(vector):
        vector.memset(zbias[:], 0.0).then_inc(dve_sem, 1)
        for t in range(_T):
            land_wait(vector, t)
            a, b = tile_ap(t)
            vector.tensor_sub(a, a, b).then_inc(dve_sem, 1)
        # fused square+sum for the DVE-owned tail tiles (data dep is
        # engine-order: their subs already ran above)
        for t in _DVE_SQ:
            a, _ = tile_ap(t)
            vector.scalar_tensor_tensor(
                a, a, 0.0, a,
                op0=mybir.AluOpType.bypass, op1=mybir.AluOpType.mult,
                accum_out=stats[:, t:t + 1],
            ).then_inc(act_sem, 1)

    @block.scalar
    def _(scalar):
        for t in range(_T):
            if t in _DVE_SQ:
                continue
            scalar.wait_ge(dve_sem, t + 2)
            a, _ = tile_ap(t)
            scalar.activation(
                a, a, mybir.ActivationFunctionType.Square,
                bias=zbias[:, 0:1], accum_out=stats[:, t:t + 1],
            ).then_inc(act_sem, 1)

    ctx.close()
    return nc


def _run(in_maps, trace=False):
    from concourse.bass_utils import run_bass_kernel_spmd

    if "nc" not in _CACHE:
        _CACHE["nc"] = _build_nc()
    return run_bass_kernel_spmd(
        _CACHE["nc"], in_maps, list(range(_NCORES)), trace=trace
    )


def _make_in_maps(input, target):
    xs = np.asarray(input, dtype=np.float32).reshape(_NCORES, _P, _FREE)
    ys = np.asarray(target, dtype=np.float32).reshape(_NCORES, _P, _FREE)
    maps = []
    for c in range(_NCORES):
        xy = np.empty((_P, 2, _FREE), dtype=np.float32)
        xy[:, 0, :] = xs[c]
        xy[:, 1, :] = ys[c]
        maps.append({"xy": xy})
    return maps


def _finish(results):
    total = 0.0
    for r in results:
        total += r["out"].astype(np.float64).sum()
    return np.array(total / _DIVISOR, dtype=np.float32)


def kernel(input, target):
    res = _run(_make_in_maps(input, target), trace=False)
    return _finish(res.results)
